# revision 19
# baseline (speedup 1.0000x reference)
"""Dice + contrastive loss on 8 Trainium2 NeuronCores.

Sharding: every input tensor [16,1,512,512] is flattened to [16, 262144]
and sharded along the *pixel* axis (32768 pixels per core).  With that
split every term of the loss becomes a local partial reduction:

  - dice:   sum(sigmoid(pred)), sum(sigmoid(pred)*gt), sum(gt)       (scalars)
  - pos:    sum((mask*(s1-s2))^2) per image                   (diag of a Gram)
  - sq1/sq2: sum(s1^2), sum(s2^2) per image                   (diag of a Gram)
  - cross:  s1 @ s2.T (16x16 Gram), contraction over pixels

Per-core SBUF layout: each input chunk is [128 partitions, 16 images x 256],
pixel n = p*256 + f so DMA moves 1 KiB contiguous runs (full BW).  The
sigmoid outputs are written straight into a Gram-pack layout
(col = t*256 + half*128 + s*16 + b) so each of the 32 PSUM-accumulated
matmuls per Gram takes one contiguous 128-col stationary block and one
contiguous 256-col moving block (float32r fast path, N=256).  The tiny
cross-core combine (a few KiB of partial sums per core) happens on the host.
"""

import os
import sys

sys.path.insert(0, "/opt/trn_rl_repo")

import numpy as np

import concourse.bass as bass
import concourse.tile as tile
from concourse import bacc, mybir
from concourse.bass_utils import run_bass_kernel_spmd

TAU = 0.1
DICE_SMOOTH = 0.1
WEIGHT = 1.0

NCORES = 8
B = 16                      # batch (images)
NPIX = 512 * 512            # pixels per image
PIX = NPIX // NCORES        # pixels per image per core = 32768
P = 128                     # partitions
F = PIX // P                # free columns per image per core = 256
G = 8                       # image groups (2 images each) for DMA/compute chunking
BG = B // G                 # images per group = 2
T = 32                      # Gram contraction chunks (each covers 8 f-columns)
S = F // T                  # sub-columns per chunk = 8

F32 = mybir.dt.float32
_GD = os.environ.get("GRAM_DT", "bf16")
F32R = {"f32r": mybir.dt.float32r, "f32": mybir.dt.float32,
        "bf16": mybir.dt.bfloat16}[_GD]
AF = mybir.ActivationFunctionType
ALU = mybir.AluOpType
AX = mybir.AxisListType


def _build_program():
    nc = bacc.Bacc("TRN2", target_bir_lowering=False, debug=False,
                   num_devices=NCORES)

    # ---- DRAM I/O (per-core shapes) ----
    # host pre-shuffles each core chunk to [P, B*F] (pixel n = p*F + f),
    # so every input DMA is a plain contiguous 2D copy
    d_in1 = nc.dram_tensor("in1", [P, B * F], F32, kind="ExternalInput")
    d_in2 = nc.dram_tensor("in2", [P, B * F], F32, kind="ExternalInput")
    d_mask = nc.dram_tensor("mask", [P, B * F], F32, kind="ExternalInput")
    d_pred = nc.dram_tensor("pred", [P, B * F], F32, kind="ExternalInput")
    d_gt = nc.dram_tensor("gt", [P, B * F], F32, kind="ExternalInput")

    o_stats = nc.dram_tensor("stats", [P, 3 * G], F32, kind="ExternalOutput")
    o_gA = nc.dram_tensor("gA", [P, 2 * P], F32, kind="ExternalOutput")
    o_gB = nc.dram_tensor("gB", [P, P], F32, kind="ExternalOutput")
    o_gC = nc.dram_tensor("gC", [P, P], F32, kind="ExternalOutput")


    with tile.TileContext(nc) as tc:
        with tc.tile_pool(name="main", bufs=1) as pool:
            # per-group input tiles so each consumer waits on exactly one DMA
            t_in1 = [pool.tile([P, BG * F], F32, name=f"t_in1_{g}", tag=f"t_in1_{g}") for g in range(G)]
            t_in2 = [pool.tile([P, BG * F], F32, name=f"t_in2_{g}", tag=f"t_in2_{g}") for g in range(G)]
            t_mask = [pool.tile([P, BG * F], F32, name=f"t_mask_{g}", tag=f"t_mask_{g}") for g in range(G)]
            t_pred = [pool.tile([P, BG * F], F32, name=f"t_pred_{g}", tag=f"t_pred_{g}") for g in range(G)]
            t_gt = [pool.tile([P, BG * F], F32, name=f"t_gt_{g}", tag=f"t_gt_{g}") for g in range(G)]
            # Gram-pack layout, col = t*256 + h*128 + s*16 + b
            # s12: h=0: s1=sig(in1), h=1: s2=sig(in2)
            s12 = pool.tile([P, 2 * B * F], F32R, tag="s12")
            # dd: h=0: d = s1-s2, h=1: dm = d*mask
            dd = pool.tile([P, 2 * B * F], F32R, tag="dd")
            t_p = pool.tile([P, B * F], F32, tag="t_p")      # sigmoid(pred)
            t_pg = pool.tile([P, B * F], F32, tag="t_pg")    # p*gt scratch
            stats = pool.tile([P, 3 * G], F32, tag="stats")
            gA_sb = pool.tile([P, 2 * P], F32, tag="gA_sb")
            gB_sb = pool.tile([P, P], F32, tag="gB_sb")
            gC_sb = pool.tile([P, P], F32, tag="gC_sb")

            with tc.tile_pool(name="psum", bufs=1, space="PSUM") as psum_pool:
                psA = psum_pool.tile([P, 2 * P], F32, tag="psA")
                psB = psum_pool.tile([P, 2 * P], F32, tag="psB")
                psC = psum_pool.tile([P, 2 * P], F32, tag="psC")

                def iview(t):
                    # group input tile [P, BG*F] -> [p, b(2), t, s]
                    return t[:].rearrange("p (b t s) -> p b t s", b=BG, s=S)

                def pview(t):
                    # Gram-pack layout [P, 2*B*F] -> [p, h, b, t, s]
                    return t[:].rearrange("p (t h s b) -> p h b t s",
                                          t=T, h=2, s=S)

                v_p, v_pg = (t_p[:].rearrange("p (b t s) -> p b t s", b=B, s=S),
                             t_pg[:].rearrange("p (b t s) -> p b t s", b=B, s=S))
                v_s12, v_dd = pview(s12), pview(dd)

                def bsl(g):
                    return slice(g * BG, (g + 1) * BG)

                dv = {"in1": d_in1.ap(), "in2": d_in2.ap(),
                      "mask": d_mask.ap(), "pred": d_pred.ap(),
                      "gt": d_gt.ap()}
                sv = {"in1": t_in1, "in2": t_in2, "mask": t_mask,
                      "pred": t_pred, "gt": t_gt}

                # ---- input DMAs: 8 chunks of 2 images per tensor ----
                # emission order = arrival priority
                CW = BG * F      # columns per group chunk
                for name in ["in1", "in2", "mask", "pred", "gt"]:
                    for g in range(G):
                        nc.sync.dma_start(
                            sv[name][g][:],
                            dv[name][:, g * CW:(g + 1) * CW])

                STAGE = os.environ.get("STAGE", "mm")
                _order = ["dma", "act", "dve", "mm"]
                _lvl = _order.index(STAGE)

                if _lvl < 1:
                    nc.vector.memset(stats[:], 0.0)

                # ---- ACT: sigmoids (pred's carries the dice sum_p accum) ----
                for g in range(_lvl >= 1 and G or 0):
                    nc.scalar.activation(v_s12[:, 0, bsl(g)], iview(t_in1[g]),
                                         AF.Sigmoid)
                for g in range(_lvl >= 1 and G or 0):
                    nc.scalar.activation(v_s12[:, 1, bsl(g)], iview(t_in2[g]),
                                         AF.Sigmoid)
                for g in range(_lvl >= 1 and G or 0):
                    nc.scalar.activation(v_p[:, bsl(g)], iview(t_pred[g]),
                                         AF.Sigmoid,
                                         accum_out=stats[:, g:g + 1])

                # ---- DVE: d = s1-s2, dm = d*mask, gt sum, p*gt sum ----
                _dve = set(os.environ.get("DVE_OPS", "d,dm,gr,pg").split(","))
                for g in range(_lvl >= 2 and "d" in _dve and G or 0):
                    nc.vector.tensor_tensor(v_dd[:, 0, bsl(g)],
                                            v_s12[:, 0, bsl(g)],
                                            v_s12[:, 1, bsl(g)], ALU.subtract)
                for g in range(_lvl >= 2 and "dm" in _dve and G or 0):
                    nc.vector.tensor_tensor(v_dd[:, 1, bsl(g)],
                                            v_dd[:, 0, bsl(g)],
                                            iview(t_mask[g]), ALU.mult)
                for g in range(_lvl >= 2 and "gr" in _dve and G or 0):
                    nc.vector.tensor_reduce(stats[:, 2 * G + g:2 * G + g + 1],
                                            t_gt[g][:], axis=AX.X,
                                            op=ALU.add)
                CWF = BG * F
                for g in range(_lvl >= 2 and "pg" in _dve and G or 0):
                    nc.vector.tensor_tensor(
                        t_pg[:, g * CWF:(g + 1) * CWF],
                        t_p[:, g * CWF:(g + 1) * CWF], t_gt[g][:], ALU.mult)
                    nc.vector.tensor_reduce(stats[:, G + g:G + g + 1],
                                            t_pg[:, g * CWF:(g + 1) * CWF],
                                            axis=AX.X, op=ALU.add)

                # ---- PE: Grams (32 PSUM-accumulated matmuls each) ----
                s12r = s12[:]
                ddr = dd[:]
                for t in range(_lvl >= 3 and T or 0):
                    st = dict(start=(t == 0), stop=(t == T - 1))
                    c0, c1, c2 = t * 2 * P, t * 2 * P + P, (t + 1) * 2 * P
                    rhs_s = s12r[:, c0:c2]           # [s1_t | s2_t], N=256
                    # A: cols 0:128 = s1.s1 (sq1), 128:256 = s1.s2 (cross)
                    nc.tensor.matmul(psA[:], s12r[:, c0:c1], rhs_s, **st)
                    # B: cols 128:256 = s2.s2 (sq2)
                    nc.tensor.matmul(psB[:], s12r[:, c1:c2], rhs_s, **st)
                    # C: cols 128:256 = dm.dm (pos)
                    nc.tensor.matmul(psC[:], ddr[:, c1:c2], ddr[:, c0:c2], **st)

                # ---- evacuate PSUM -> SBUF -> DRAM ----
                if _lvl >= 3:
                    nc.scalar.copy(gA_sb[:], psA[:])
                    nc.vector.tensor_copy(gB_sb[:], psB[:, P:2 * P])
                    nc.vector.tensor_copy(gC_sb[:], psC[:, P:2 * P])
                else:
                    nc.vector.memset(gA_sb[:], 0.0)
                    nc.vector.memset(gB_sb[:], 0.0)
                    nc.vector.memset(gC_sb[:], 0.0)

                nc.sync.dma_start(o_stats.ap(), stats[:])
                nc.sync.dma_start(o_gA.ap(), gA_sb[:])
                nc.sync.dma_start(o_gB.ap(), gB_sb[:])
                nc.sync.dma_start(o_gC.ap(), gC_sb[:])

    nc.compile()
    return nc


_NC_CACHE = None


def _get_program():
    global _NC_CACHE
    if _NC_CACHE is None:
        _NC_CACHE = _build_program()
    return _NC_CACHE


def _shard_inputs(pred_labeled, gt_labeled, input1, input2, mask):
    flat = {
        "pred": np.asarray(pred_labeled, dtype=np.float32).reshape(B, NPIX),
        "gt": np.asarray(gt_labeled, dtype=np.float32).reshape(B, NPIX),
        "in1": np.asarray(input1, dtype=np.float32).reshape(B, NPIX),
        "in2": np.asarray(input2, dtype=np.float32).reshape(B, NPIX),
        "mask": np.asarray(mask, dtype=np.float32).reshape(B, NPIX),
    }
    in_maps = []
    for k in range(NCORES):
        sl = slice(k * PIX, (k + 1) * PIX)
        in_maps.append({
            n: np.ascontiguousarray(
                a[:, sl].reshape(B, P, F).transpose(1, 0, 2).reshape(P, B * F))
            for n, a in flat.items()})
    return in_maps


def _block_diag_sum(gmat):
    # [128, 128] with rows (s*16+b1), cols (s*16+b2) -> sum_s of [16,16] blocks
    g = gmat.reshape(S, B, S, B)
    return np.einsum("sbsc->bc", g)


def _combine(results):
    sum_p = sum_pg = sum_g = 0.0
    g1 = np.zeros((B, B), np.float64)
    cr = np.zeros((B, B), np.float64)
    g2 = np.zeros((B, B), np.float64)
    pc = np.zeros((B, B), np.float64)
    for r in results:
        st = r["stats"].astype(np.float64)
        sum_p += st[:, 0:G].sum()
        sum_pg += st[:, G:2 * G].sum()
        sum_g += st[:, 2 * G:3 * G].sum()
        gA = r["gA"].astype(np.float64)
        g1 += _block_diag_sum(gA[:, :P])
        cr += _block_diag_sum(gA[:, P:])
        g2 += _block_diag_sum(r["gB"].astype(np.float64))
        pc += _block_diag_sum(r["gC"].astype(np.float64))

    dice = 1.0 - (2.0 * sum_pg + DICE_SMOOTH) / (sum_p + sum_g + DICE_SMOOTH)

    n = float(NPIX)
    sq1 = np.diag(g1) / n
    sq2 = np.diag(g2) / n
    cross = cr / n
    pos_mse = np.diag(pc) / n

    sim_pos = np.exp(-pos_mse / TAU)
    mse = sq1[:, None] + sq2[None, :] - 2.0 * cross
    sim = np.exp(-mse / TAU)
    sim_neg = (sim * (1.0 - np.eye(B))).sum(axis=1)
    loss_c = float(np.mean(-np.log(sim_pos / (sim_pos + sim_neg))))
    total = dice + WEIGHT * loss_c
    return (np.float32(total), np.float32(dice), 0.0, np.float32(loss_c))


def kernel(pred_labeled, gt_labeled, input1, input2, mask):
    nc = _get_program()
    in_maps = _shard_inputs(pred_labeled, gt_labeled, input1, input2, mask)
    res = run_bass_kernel_spmd(nc, in_maps, core_ids=list(range(NCORES)),
                               trace=bool(int(os.environ.get("KERNEL_TRACE", "0"))))
    out = _combine(res.results)
    if res.exec_time_ns is not None:
        print(f"HW exec time: {res.exec_time_ns} ns")
    return out


# revision 20
# speedup vs baseline: 1.7618x; 1.7618x over previous
"""Dice + contrastive loss on 8 Trainium2 NeuronCores.

Sharding: every input tensor [16,1,512,512] is flattened to [16, 262144]
and sharded along the *pixel* axis (32768 pixels per core).  With that
split every term of the loss becomes a local partial reduction:

  - dice:   sum(sigmoid(pred)), sum(sigmoid(pred)*gt), sum(gt)       (scalars)
  - pos:    sum((mask*(s1-s2))^2) per image                   (diag of a Gram)
  - sq1/sq2: sum(s1^2), sum(s2^2) per image                   (diag of a Gram)
  - cross:  s1 @ s2.T (16x16 Gram), contraction over pixels

Per-core SBUF layout: each input chunk is [128 partitions, 16 images x 256],
pixel n = p*256 + f so DMA moves 1 KiB contiguous runs (full BW).  The
sigmoid outputs are written straight into a Gram-pack layout
(col = t*256 + half*128 + s*16 + b) so each of the 32 PSUM-accumulated
matmuls per Gram takes one contiguous 128-col stationary block and one
contiguous 256-col moving block (float32r fast path, N=256).  The tiny
cross-core combine (a few KiB of partial sums per core) happens on the host.
"""

import os
import sys

sys.path.insert(0, "/opt/trn_rl_repo")

import numpy as np

import concourse.bass as bass
import concourse.tile as tile
from concourse import bacc, mybir
from concourse.bass_utils import run_bass_kernel_spmd

TAU = 0.1
DICE_SMOOTH = 0.1
WEIGHT = 1.0

NCORES = 8
B = 16                      # batch (images)
NPIX = 512 * 512            # pixels per image
PIX = NPIX // NCORES        # pixels per image per core = 32768
P = 128                     # partitions
F = PIX // P                # free columns per image per core = 256
G = 8                       # image groups (2 images each) for DMA/compute chunking
BG = B // G                 # images per group = 2
T = 32                      # Gram contraction chunks (each covers 8 f-columns)
S = F // T                  # sub-columns per chunk = 8

F32 = mybir.dt.float32
_GD = os.environ.get("GRAM_DT", "bf16")
F32R = {"f32r": mybir.dt.float32r, "f32": mybir.dt.float32,
        "bf16": mybir.dt.bfloat16}[_GD]
AF = mybir.ActivationFunctionType
ALU = mybir.AluOpType
AX = mybir.AxisListType


def _build_program():
    nc = bacc.Bacc("TRN2", target_bir_lowering=False, debug=False,
                   num_devices=NCORES)

    # ---- DRAM I/O (per-core shapes) ----
    # host pre-shuffles each core chunk to [P, B*F] (pixel n = p*F + f),
    # so every input DMA is a plain contiguous 2D copy
    d_in1 = nc.dram_tensor("in1", [P, B * F], F32, kind="ExternalInput")
    d_in2 = nc.dram_tensor("in2", [P, B * F], F32, kind="ExternalInput")
    d_mask = nc.dram_tensor("mask", [P, B * F], F32, kind="ExternalInput")
    d_pred = nc.dram_tensor("pred", [P, B * F], F32, kind="ExternalInput")
    d_gt = nc.dram_tensor("gt", [P, B * F], F32, kind="ExternalInput")

    o_stats = nc.dram_tensor("stats", [P, 3 * G], F32, kind="ExternalOutput")
    o_gA = nc.dram_tensor("gA", [P, 2 * P], F32, kind="ExternalOutput")
    o_gB = nc.dram_tensor("gB", [P, P], F32, kind="ExternalOutput")
    o_gC = nc.dram_tensor("gC", [P, P], F32, kind="ExternalOutput")


    with tile.TileContext(nc) as tc:
        with tc.tile_pool(name="main", bufs=1) as pool:
            # per-group input tiles so each consumer waits on exactly one DMA
            t_in1 = [pool.tile([P, BG * F], F32, name=f"t_in1_{g}", tag=f"t_in1_{g}") for g in range(G)]
            t_in2 = [pool.tile([P, BG * F], F32, name=f"t_in2_{g}", tag=f"t_in2_{g}") for g in range(G)]
            t_mask = [pool.tile([P, BG * F], F32, name=f"t_mask_{g}", tag=f"t_mask_{g}") for g in range(G)]
            t_pred = [pool.tile([P, BG * F], F32, name=f"t_pred_{g}", tag=f"t_pred_{g}") for g in range(G)]
            t_gt = [pool.tile([P, BG * F], F32, name=f"t_gt_{g}", tag=f"t_gt_{g}") for g in range(G)]
            # Gram-pack layout, col = t*256 + h*128 + s*16 + b
            # s12: h=0: s1=sig(in1), h=1: s2=sig(in2)
            s12 = pool.tile([P, 2 * B * F], F32R, tag="s12")
            # dd: h=0: d = s1-s2, h=1: dm = d*mask
            dd = pool.tile([P, 2 * B * F], F32R, tag="dd")
            t_p = pool.tile([P, B * F], F32, tag="t_p")      # sigmoid(pred)
            t_pg = pool.tile([P, B * F], F32, tag="t_pg")    # p*gt scratch
            stats = pool.tile([P, 3 * G], F32, tag="stats")
            gA_sb = pool.tile([P, 2 * P], F32, tag="gA_sb")
            gB_sb = pool.tile([P, P], F32, tag="gB_sb")
            gC_sb = pool.tile([P, P], F32, tag="gC_sb")

            with tc.tile_pool(name="psum", bufs=1, space="PSUM") as psum_pool:
                psA = psum_pool.tile([P, 2 * P], F32, tag="psA")
                psB = psum_pool.tile([P, 2 * P], F32, tag="psB")
                psC = psum_pool.tile([P, 2 * P], F32, tag="psC")

                def tview(t, n=128):
                    # [P, X] -> [p, tchunk, n] with n contiguous cols per chunk
                    return t[:].rearrange("p (t c) -> p t c", c=n)

                # s12/dd: [p, t(32), h(2), c(128)] - h blocks interleaved per chunk
                v_s12 = s12[:].rearrange("p (t h c) -> p t h c", h=2, c=P)
                v_dd = dd[:].rearrange("p (t h c) -> p t h c", h=2, c=P)
                TG = T // G      # t-chunks per group = 4

                def tsl(g):
                    return slice(g * TG, (g + 1) * TG)

                dv = {"in1": d_in1.ap(), "in2": d_in2.ap(),
                      "mask": d_mask.ap(), "pred": d_pred.ap(),
                      "gt": d_gt.ap()}
                sv = {"in1": t_in1, "in2": t_in2, "mask": t_mask,
                      "pred": t_pred, "gt": t_gt}

                # ---- input DMAs: 8 chunks of 2 images per tensor ----
                # emission order = arrival priority
                CW = BG * F      # columns per group chunk
                for name in ["in1", "in2", "mask", "pred", "gt"]:
                    for g in range(G):
                        nc.sync.dma_start(
                            sv[name][g][:],
                            dv[name][:, g * CW:(g + 1) * CW])

                STAGE = os.environ.get("STAGE", "mm")
                _order = ["dma", "act", "dve", "mm"]
                _lvl = _order.index(STAGE)

                if _lvl < 1:
                    nc.vector.memset(stats[:], 0.0)

                # ---- ACT: sigmoids (pred's carries the dice sum_p accum) ----
                CWF = BG * F     # 512 cols per group chunk
                for g in range(_lvl >= 1 and G or 0):
                    nc.scalar.activation(v_s12[:, tsl(g), 0, :],
                                         tview(t_in1[g]), AF.Sigmoid)
                for g in range(_lvl >= 1 and G or 0):
                    nc.scalar.activation(v_s12[:, tsl(g), 1, :],
                                         tview(t_in2[g]), AF.Sigmoid)
                for g in range(_lvl >= 1 and G or 0):
                    nc.scalar.activation(t_p[:, g * CWF:(g + 1) * CWF],
                                         t_pred[g][:], AF.Sigmoid,
                                         accum_out=stats[:, g:g + 1])

                # ---- DVE: d = s1-s2, dm = d*mask, gt sum, p*gt sum ----
                _dve = set(os.environ.get("DVE_OPS", "d,dm,gr,pg").split(","))
                for g in range(_lvl >= 2 and "d" in _dve and G or 0):
                    nc.vector.tensor_tensor(v_dd[:, tsl(g), 0, :],
                                            v_s12[:, tsl(g), 0, :],
                                            v_s12[:, tsl(g), 1, :], ALU.subtract)
                for g in range(_lvl >= 2 and "dm" in _dve and G or 0):
                    nc.vector.tensor_tensor(v_dd[:, tsl(g), 1, :],
                                            v_dd[:, tsl(g), 0, :],
                                            tview(t_mask[g]), ALU.mult)
                for g in range(_lvl >= 2 and "gr" in _dve and G or 0):
                    nc.vector.tensor_reduce(stats[:, 2 * G + g:2 * G + g + 1],
                                            t_gt[g][:], axis=AX.X,
                                            op=ALU.add)
                for g in range(_lvl >= 2 and "pg" in _dve and G or 0):
                    nc.vector.tensor_tensor(
                        t_pg[:, g * CWF:(g + 1) * CWF],
                        t_p[:, g * CWF:(g + 1) * CWF], t_gt[g][:], ALU.mult)
                    nc.vector.tensor_reduce(stats[:, G + g:G + g + 1],
                                            t_pg[:, g * CWF:(g + 1) * CWF],
                                            axis=AX.X, op=ALU.add)

                # ---- PE: Grams (32 PSUM-accumulated matmuls each) ----
                s12r = s12[:]
                ddr = dd[:]
                for t in range(_lvl >= 3 and T or 0):
                    st = dict(start=(t == 0), stop=(t == T - 1))
                    c0, c1, c2 = t * 2 * P, t * 2 * P + P, (t + 1) * 2 * P
                    rhs_s = s12r[:, c0:c2]           # [s1_t | s2_t], N=256
                    # A: cols 0:128 = s1.s1 (sq1), 128:256 = s1.s2 (cross)
                    nc.tensor.matmul(psA[:], s12r[:, c0:c1], rhs_s, **st)
                    # B: cols 128:256 = s2.s2 (sq2)
                    nc.tensor.matmul(psB[:], s12r[:, c1:c2], rhs_s, **st)
                    # C: cols 128:256 = dm.dm (pos)
                    nc.tensor.matmul(psC[:], ddr[:, c1:c2], ddr[:, c0:c2], **st)

                # ---- evacuate PSUM -> SBUF -> DRAM ----
                if _lvl >= 3:
                    nc.scalar.copy(gA_sb[:], psA[:])
                    nc.vector.tensor_copy(gB_sb[:], psB[:, P:2 * P])
                    nc.vector.tensor_copy(gC_sb[:], psC[:, P:2 * P])
                else:
                    nc.vector.memset(gA_sb[:], 0.0)
                    nc.vector.memset(gB_sb[:], 0.0)
                    nc.vector.memset(gC_sb[:], 0.0)

                nc.sync.dma_start(o_stats.ap(), stats[:])
                nc.sync.dma_start(o_gA.ap(), gA_sb[:])
                nc.sync.dma_start(o_gB.ap(), gB_sb[:])
                nc.sync.dma_start(o_gC.ap(), gC_sb[:])

    nc.compile()
    return nc


_NC_CACHE = None


def _get_program():
    global _NC_CACHE
    if _NC_CACHE is None:
        _NC_CACHE = _build_program()
    return _NC_CACHE


def _shard_inputs(pred_labeled, gt_labeled, input1, input2, mask):
    flat = {
        "pred": np.asarray(pred_labeled, dtype=np.float32).reshape(B, NPIX),
        "gt": np.asarray(gt_labeled, dtype=np.float32).reshape(B, NPIX),
        "in1": np.asarray(input1, dtype=np.float32).reshape(B, NPIX),
        "in2": np.asarray(input2, dtype=np.float32).reshape(B, NPIX),
        "mask": np.asarray(mask, dtype=np.float32).reshape(B, NPIX),
    }
    def nat(a, sl):   # natural: [P, (b f)]
        return np.ascontiguousarray(
            a[:, sl].reshape(B, P, F).transpose(1, 0, 2).reshape(P, B * F))

    def pack(a, sl):  # Gram pack: [P, (t s b)]
        return np.ascontiguousarray(
            a[:, sl].reshape(B, P, T, S).transpose(1, 2, 3, 0)
            .reshape(P, B * F))

    in_maps = []
    for k in range(NCORES):
        sl = slice(k * PIX, (k + 1) * PIX)
        in_maps.append({
            "pred": nat(flat["pred"], sl), "gt": nat(flat["gt"], sl),
            "in1": pack(flat["in1"], sl), "in2": pack(flat["in2"], sl),
            "mask": pack(flat["mask"], sl)})
    return in_maps


def _block_diag_sum(gmat):
    # [128, 128] with rows (s*16+b1), cols (s*16+b2) -> sum_s of [16,16] blocks
    g = gmat.reshape(S, B, S, B)
    return np.einsum("sbsc->bc", g)


def _combine(results):
    sum_p = sum_pg = sum_g = 0.0
    g1 = np.zeros((B, B), np.float64)
    cr = np.zeros((B, B), np.float64)
    g2 = np.zeros((B, B), np.float64)
    pc = np.zeros((B, B), np.float64)
    for r in results:
        st = r["stats"].astype(np.float64)
        sum_p += st[:, 0:G].sum()
        sum_pg += st[:, G:2 * G].sum()
        sum_g += st[:, 2 * G:3 * G].sum()
        gA = r["gA"].astype(np.float64)
        g1 += _block_diag_sum(gA[:, :P])
        cr += _block_diag_sum(gA[:, P:])
        g2 += _block_diag_sum(r["gB"].astype(np.float64))
        pc += _block_diag_sum(r["gC"].astype(np.float64))

    dice = 1.0 - (2.0 * sum_pg + DICE_SMOOTH) / (sum_p + sum_g + DICE_SMOOTH)

    n = float(NPIX)
    sq1 = np.diag(g1) / n
    sq2 = np.diag(g2) / n
    cross = cr / n
    pos_mse = np.diag(pc) / n

    sim_pos = np.exp(-pos_mse / TAU)
    mse = sq1[:, None] + sq2[None, :] - 2.0 * cross
    sim = np.exp(-mse / TAU)
    sim_neg = (sim * (1.0 - np.eye(B))).sum(axis=1)
    loss_c = float(np.mean(-np.log(sim_pos / (sim_pos + sim_neg))))
    total = dice + WEIGHT * loss_c
    return (np.float32(total), np.float32(dice), 0.0, np.float32(loss_c))


def kernel(pred_labeled, gt_labeled, input1, input2, mask):
    nc = _get_program()
    in_maps = _shard_inputs(pred_labeled, gt_labeled, input1, input2, mask)
    res = run_bass_kernel_spmd(nc, in_maps, core_ids=list(range(NCORES)),
                               trace=bool(int(os.environ.get("KERNEL_TRACE", "0"))))
    out = _combine(res.results)
    if res.exec_time_ns is not None:
        print(f"HW exec time: {res.exec_time_ns} ns")
    return out
